# revision 25
# baseline (speedup 1.0000x reference)
"""Trainium2 Bass kernel for EnhancedLocalAttentionWithGQA.

Problem (hardcoded): B=2, L=4096, C=2048, H=16 heads, D=128, G=2 kv groups,
window W=256 with stride 128 (50% overlap).

Key observation: the reference computes NW=31 overlapping windows but the
final output slice [:, :L] keeps only windows 0..15 (16 windows x 256 rows
= 4096 rows).  Window n's output rows [n*256,(n+1)*256) come from queries /
keys / values at input positions [n*128, n*128+256).  So only x positions
0..2175 feed QKV, and each window is an independent 256x256 attention.

Sharding (8 cores): core c -> batch b=c//4, quarter p=c%4, i.e. 4 windows
(global windows 4p..4p+3), input positions [512p, 512p+640), output rows
[1024p, 1024p+1024) of batch b.  No collectives; host concatenates rows.

Per-core pipeline (bf16 matmuls, fp32 PSUM):
  resident xT -> Q^T per head (feat-major), K^T per group, V seq-major;
  per (window,head): S^T = K^T_chunk.T @ Q^T, P^T = exp(scale*S^T) [ACT],
  O^T += V_chunk.T @ P^T, colsum = ones.T @ P^T, broadcast colsum via a
  K=1 matmul, reciprocal_approx_fast on the broadcast, normalize -> O^T
  resident; out = O^T.T @ Wo + bo streamed per 512-col block.

All weights are host-pretiled so every DMA is a single large contiguous
transfer.
"""

import numpy as np
import ml_dtypes

import concourse.bacc as bacc
import concourse.tile as tile
from concourse import mybir
from concourse.bass_utils import run_bass_kernel_spmd

F32 = mybir.dt.float32
BF16 = mybir.dt.bfloat16

B = 2
L = 4096
C = 2048          # embed dim
H = 16            # heads
G = 2             # kv groups
D = 128           # head dim
KV = G * D        # 256
NWL = 4           # windows per core
S = NWL * 128 + 128   # 640 input positions per core
OUT_ROWS = NWL * 256  # 1024 output rows per core
KC = C // 128     # 16 contraction chunks
NT = 4            # out-proj 512-col tiles
SC_OUT = OUT_ROWS // 128
SCALE = 1.0 / float(np.sqrt(D))
N_CORES = 8

_CACHE = {}


def _build():
    nc = bacc.Bacc(None, target_bir_lowering=False)

    # host-pretiled layouts (see kernel() for the numpy side)
    xT_d = nc.dram_tensor("xT", [128, KC, S], BF16, kind="ExternalInput")
    wq_d = nc.dram_tensor("Wq", [H, 128, KC, 128], BF16, kind="ExternalInput")
    wk_d = nc.dram_tensor("Wk", [G, 128, KC, 128], BF16, kind="ExternalInput")
    wv_d = nc.dram_tensor("Wv", [128, KC, KV], BF16, kind="ExternalInput")
    wo_d = nc.dram_tensor("Wo", [NT, 128, KC, 512], BF16, kind="ExternalInput")
    bq_d = nc.dram_tensor("bq", [C], F32, kind="ExternalInput")
    bk_d = nc.dram_tensor("bk", [KV], F32, kind="ExternalInput")
    bv_d = nc.dram_tensor("bv", [KV], F32, kind="ExternalInput")
    bo_d = nc.dram_tensor("bo", [C], F32, kind="ExternalInput")
    out_d = nc.dram_tensor("out", [NT, SC_OUT, 128, 512], F32,
                           kind="ExternalOutput")

    with tile.TileContext(nc) as tc:
        with (
            tc.tile_pool(name="res", bufs=1) as res,
            tc.tile_pool(name="wqs", bufs=3) as wqs,
            tc.tile_pool(name="wos", bufs=2) as wos,
            tc.tile_pool(name="work", bufs=3) as work,
            tc.tile_pool(name="norm", bufs=4) as norm,
            tc.tile_pool(name="dram", bufs=8, space="DRAM") as dpool,
        ):
            # ---------- resident loads ----------
            # first head's weights first so the PE can start ASAP, then x
            # in 4 chunks; small/secondary loads go on the gpsimd queue.
            wq0s = [res.tile([128, 4, 128], BF16, tag=f"wq0{j}", name=f"wq0{j}")
                    for j in range(4)]
            xtq = [res.tile([128, 2, S], BF16, tag=f"xt{j}", name=f"xt{j}")
                   for j in range(8)]
            nc.sync.dma_start(out=wq0s[0], in_=wq_d[0][:, 0:4, :])
            nc.sync.dma_start(out=xtq[0], in_=xT_d[:, 0:2, :])
            nc.sync.dma_start(out=xtq[1], in_=xT_d[:, 2:4, :])
            for j in range(1, 4):
                nc.sync.dma_start(out=wq0s[j], in_=wq_d[0][:, j * 4:(j + 1) * 4, :])
            for j in range(2, 8):
                nc.sync.dma_start(out=xtq[j], in_=xT_d[:, j * 2:(j + 1) * 2, :])

            def xts(kc):
                return xtq[kc // 2][:, kc % 2, :]

            wv_t = res.tile([128, KC, KV], BF16, tag="wv", name="wv")
            nc.gpsimd.dma_start(out=wv_t, in_=wv_d[:, :, :])
            kw = [res.tile([128, KC, 128], BF16, tag=f"kw{g}", name=f"kw{g}")
                  for g in range(G)]
            for g in range(G):
                nc.gpsimd.dma_start(out=kw[g], in_=wk_d[g])

            bq_sb = res.tile([128, H], F32, tag="bq", name="bq")
            nc.gpsimd.dma_start(out=bq_sb, in_=bq_d[:].rearrange("(h p) -> p h", p=128))
            bk_sb = res.tile([128, G], F32, tag="bk", name="bk")
            nc.gpsimd.dma_start(out=bk_sb, in_=bk_d[:].rearrange("(g p) -> p g", p=128))
            bv_bc = res.tile([128, KV], F32, tag="bvbc", name="bvbc")
            nc.gpsimd.dma_start(out=bv_bc,
                              in_=bv_d[:].unsqueeze(0).to_broadcast((128, KV)))
            bo_bc = res.tile([128, C], F32, tag="bobc", name="bobc")
            nc.gpsimd.dma_start(out=bo_bc,
                              in_=bo_d[:].unsqueeze(0).to_broadcast((128, C)))

            ones = res.tile([128, 1], BF16, tag="ones", name="ones")
            nc.vector.memset(ones, 1.0)
            ones_r = res.tile([1, 128], BF16, tag="ones_r", name="ones_r")
            nc.vector.memset(ones_r, 1.0)

            # paired Q storage: qp[g*4+j] holds heads (g+4j, g+4j+2)
            qp = [res.tile([128, 2, S], BF16, tag=f"qp{i}", name=f"qp{i}")
                  for i in range(8)]

            def q_slot(h):
                g, k = h % G, h // G
                return qp[g * 4 + k // 2][:, k % 2, :]
            kt = [res.tile([128, S], BF16, tag=f"kt{g}", name=f"kt{g}")
                  for g in range(G)]
            vt = [res.tile([128, KV], BF16, tag=f"vt{sc}", name=f"vt{sc}")
                  for sc in range(S // 128)]
            ot = [res.tile([128, 2, OUT_ROWS], BF16, tag=f"ot{i}", name=f"ot{i}")
                  for i in range(8)]

            def ot_slot(h):
                g, k = h % G, h // G
                return ot[g * 4 + k // 2][:, k % 2, :]

            NA, NB = 320, 320  # free split of S=640 (psum bank = 512 f32)

            # ---------- projections ----------
            with tc.tile_pool(name="psA", bufs=2, space="PSUM") as psA:
                for h in range(H):
                    if h > 0:
                        wq_t = wqs.tile([128, KC, 128], BF16, tag="wq", name="wq")
                        nc.sync.dma_start(out=wq_t, in_=wq_d[h])

                    def wql(kc, h=h, wq_t=(None if h == 0 else wq_t)):
                        if h == 0:
                            return wq0s[kc // 4][:, kc % 4, :]
                        return wq_t[:, kc, :]
                    pa = psA.tile([128, NA], F32, tag="qa", name="qa")
                    pb = psA.tile([128, NB], F32, tag="qb", name="qb")
                    for kc in range(KC):
                        nc.tensor.matmul(pa, lhsT=wql(kc),
                                         rhs=xts(kc)[:, 0:NA],
                                         start=(kc == 0), stop=(kc == KC - 1))
                    for kc in range(KC):
                        nc.tensor.matmul(pb, lhsT=wql(kc),
                                         rhs=xts(kc)[:, NA:S],
                                         start=(kc == 0), stop=(kc == KC - 1))
                    nc.scalar.activation(q_slot(h)[:, 0:NA], pa,
                                         mybir.ActivationFunctionType.Identity,
                                         bias=bq_sb[:, h:h + 1])
                    nc.scalar.activation(q_slot(h)[:, NA:S], pb,
                                         mybir.ActivationFunctionType.Identity,
                                         bias=bq_sb[:, h:h + 1])

                for g in range(G):
                    pa = psA.tile([128, NA], F32, tag="qa", name="qa")
                    pb = psA.tile([128, NB], F32, tag="qb", name="qb")
                    for kc in range(KC):
                        nc.tensor.matmul(pa, lhsT=kw[g][:, kc, :],
                                         rhs=xts(kc)[:, 0:NA],
                                         start=(kc == 0), stop=(kc == KC - 1))
                    for kc in range(KC):
                        nc.tensor.matmul(pb, lhsT=kw[g][:, kc, :],
                                         rhs=xts(kc)[:, NA:S],
                                         start=(kc == 0), stop=(kc == KC - 1))
                    nc.scalar.activation(kt[g][:, 0:NA], pa,
                                         mybir.ActivationFunctionType.Identity,
                                         bias=bk_sb[:, g:g + 1])
                    nc.scalar.activation(kt[g][:, NA:S], pb,
                                         mybir.ActivationFunctionType.Identity,
                                         bias=bk_sb[:, g:g + 1])

                for sc in range(S // 128):
                    pv = psA.tile([128, KV], F32, tag="vp", name="vp")
                    for kc in range(KC):
                        nc.tensor.matmul(
                            pv, lhsT=xts(kc)[:, sc * 128:(sc + 1) * 128],
                            rhs=wv_t[:, kc, :],
                            start=(kc == 0), stop=(kc == KC - 1))
                    nc.vector.tensor_add(vt[sc], pv, bv_bc)

            # ---------- windowed attention (paired heads, pipelined) ----------
            # A "pair" = 2 heads of the same kv group -> N=512 matmuls over
            # both heads at once; 3-stage software pipeline keeps the PE from
            # waiting on ACT/DVE round-trips.
            pairs = [(w, g, j) for w in range(NWL)
                     for g in range(G) for j in range(4)]
            NP = len(pairs)
            state = {}

            def stage_a(i):
                w, g, j = pairs[i]
                q0 = w * 128
                qpt = qp[g * 4 + j]
                pt = work.tile([128, 2, 2, 256], BF16, tag="pt", name="pt")
                for kc in range(2):
                    st = psB.tile([128, 512], F32, tag="st", name="st")
                    nc.tensor.matmul(
                        st, lhsT=kt[g][:, q0 + kc * 128:q0 + (kc + 1) * 128],
                        rhs=qpt[:, :, q0:q0 + 256],
                        start=True, stop=True)
                    nc.scalar.activation(pt[:, kc, :, :], st,
                                         mybir.ActivationFunctionType.Exp,
                                         scale=SCALE)
                state[i] = [pt]

            def stage_b(i):
                w, g, j = pairs[i]
                (pt,) = state[i]
                ob = psB2.tile([128, 512], F32, tag="ob", name="ob")
                for kc in range(2):
                    nc.tensor.matmul(ob,
                                     lhsT=vt[w + kc][:, g * 128:(g + 1) * 128],
                                     rhs=pt[:, kc, :, :],
                                     start=(kc == 0), stop=(kc == 1))
                cs = psB3.tile([1, 512], F32, tag="cs", name="cs")
                for kc in range(2):
                    nc.tensor.matmul(cs, lhsT=ones, rhs=pt[:, kc, :, :],
                                     start=(kc == 0), stop=(kc == 1))
                csb = norm.tile([1, 512], BF16, tag="csb", name="csb")
                nc.scalar.copy(csb, cs)
                state[i] = [ob, csb]

            def stage_c(i):
                w, g, j = pairs[i]
                h0, h1 = g + 4 * j, g + 4 * j + 2
                ob, csb = state.pop(i)
                bc = psB3.tile([128, 512], F32, tag="bc", name="bc")
                nc.tensor.matmul(bc, lhsT=ones_r, rhs=csb,
                                 start=True, stop=True)
                bcr = norm.tile([128, 512], F32, tag="bcr", name="bcr")
                nc.vector.reciprocal_approx_fast(out=bcr, in_=bc)
                nc.vector.tensor_mul(
                    ot[g * 4 + j][:, :, w * 256:(w + 1) * 256], ob, bcr)

            with (
                tc.tile_pool(name="psB", bufs=4, space="PSUM") as psB,
                tc.tile_pool(name="psB2", bufs=2, space="PSUM") as psB2,
                tc.tile_pool(name="psB3", bufs=1, space="PSUM") as psB3,
            ):
                for i in range(NP + 2):
                    if 1 <= i < NP + 1:
                        stage_b(i - 1)
                    if i < NP:
                        stage_a(i)
                    if 2 <= i:
                        stage_c(i - 2)

            # ---------- out-projection ----------
            with tc.tile_pool(name="psC", bufs=4, space="PSUM") as psC:
                for nt in range(NT):
                    wo_t = wos.tile([128, KC, 512], BF16, tag="wo", name="wo")
                    nc.sync.dma_start(out=wo_t, in_=wo_d[nt])
                    for sc in range(SC_OUT):
                        po = psC.tile([128, 512], F32, tag="op", name="op")
                        for fc in range(KC):
                            nc.tensor.matmul(
                                po, lhsT=ot_slot(fc)[:, sc * 128:(sc + 1) * 128],
                                rhs=wo_t[:, fc, :],
                                start=(fc == 0), stop=(fc == KC - 1))
                        osb = work.tile([128, 512], F32, tag="osb", name="osb")
                        nc.vector.tensor_add(osb, po,
                                             bo_bc[:, nt * 512:(nt + 1) * 512])
                        nc.sync.dma_start(out=out_d[nt, sc], in_=osb)

    nc.compile()
    return nc


def _get_nc():
    if "nc" not in _CACHE:
        _CACHE["nc"] = _build()
    return _CACHE["nc"]


def _prep_weights(Wq, bq, Wk, bk, Wv, bv, Wo, bo):
    bf16 = ml_dtypes.bfloat16
    f32 = lambda a: np.ascontiguousarray(np.asarray(a, dtype=np.float32))
    wq = np.asarray(Wq, np.float32).reshape(KC, 128, H, 128)
    wq = np.ascontiguousarray(wq.transpose(2, 1, 0, 3)).astype(bf16)  # (H,p,kc,f)
    wk = np.asarray(Wk, np.float32).reshape(KC, 128, G, 128)
    wk = np.ascontiguousarray(wk.transpose(2, 1, 0, 3)).astype(bf16)  # (G,p,kc,f)
    wv = np.asarray(Wv, np.float32).reshape(KC, 128, KV)
    wv = np.ascontiguousarray(wv.transpose(1, 0, 2)).astype(bf16)     # (p,kc,f)
    wo = np.asarray(Wo, np.float32).reshape(KC, 128, NT, 512)
    wo = np.ascontiguousarray(wo.transpose(2, 1, 0, 3)).astype(bf16)  # (NT,p,kc,f)
    return {
        "Wq": wq, "Wk": wk, "Wv": wv, "Wo": wo,
        "bq": f32(bq), "bk": f32(bk), "bv": f32(bv), "bo": f32(bo),
    }


def _prep_in_maps(x, weights):
    bf16 = ml_dtypes.bfloat16
    in_maps = []
    for c in range(N_CORES):
        b, p = divmod(c, 4)
        xs = np.asarray(x[b, 512 * p:512 * p + S, :], np.float32)
        xT = np.ascontiguousarray(xs.T.reshape(KC, 128, S).transpose(1, 0, 2))
        in_maps.append(dict(weights, xT=xT.astype(bf16)))
    return in_maps


def kernel(x, Wq, bq, Wk, bk, Wv, bv, Wo, bo, **_):
    x = np.asarray(x, dtype=np.float32)
    weights = _prep_weights(Wq, bq, Wk, bk, Wv, bv, Wo, bo)
    in_maps = _prep_in_maps(x, weights)

    nc = _get_nc()
    res = run_bass_kernel_spmd(nc, in_maps, core_ids=list(range(N_CORES)))

    out = np.empty((B, L, C), dtype=np.float32)
    for c in range(N_CORES):
        b, p = divmod(c, 4)
        blk = res.results[c]["out"]  # (NT, SC_OUT, 128, 512)
        rows = blk.transpose(1, 2, 0, 3).reshape(OUT_ROWS, C)
        out[b, 1024 * p:1024 * p + OUT_ROWS, :] = rows
    return out


# revision 26
# speedup vs baseline: 1.0113x; 1.0113x over previous
"""Trainium2 Bass kernel for EnhancedLocalAttentionWithGQA.

Problem (hardcoded): B=2, L=4096, C=2048, H=16 heads, D=128, G=2 kv groups,
window W=256 with stride 128 (50% overlap).

Key observation: the reference computes NW=31 overlapping windows but the
final output slice [:, :L] keeps only windows 0..15 (16 windows x 256 rows
= 4096 rows).  Window n's output rows [n*256,(n+1)*256) come from queries /
keys / values at input positions [n*128, n*128+256).  So only x positions
0..2175 feed QKV, and each window is an independent 256x256 attention.

Sharding (8 cores): core c -> batch b=c//4, quarter p=c%4, i.e. 4 windows
(global windows 4p..4p+3), input positions [512p, 512p+640), output rows
[1024p, 1024p+1024) of batch b.  No collectives; host concatenates rows.

Per-core pipeline (bf16 matmuls, fp32 PSUM):
  resident xT -> Q^T per head (feat-major), K^T per group, V seq-major;
  per (window,head): S^T = K^T_chunk.T @ Q^T, P^T = exp(scale*S^T) [ACT],
  O^T += V_chunk.T @ P^T, colsum = ones.T @ P^T, broadcast colsum via a
  K=1 matmul, reciprocal_approx_fast on the broadcast, normalize -> O^T
  resident; out = O^T.T @ Wo + bo streamed per 512-col block.

All weights are host-pretiled so every DMA is a single large contiguous
transfer.
"""

import numpy as np
import ml_dtypes

import concourse.bacc as bacc
import concourse.tile as tile
from concourse import mybir
from concourse.bass_utils import run_bass_kernel_spmd

F32 = mybir.dt.float32
BF16 = mybir.dt.bfloat16

B = 2
L = 4096
C = 2048          # embed dim
H = 16            # heads
G = 2             # kv groups
D = 128           # head dim
KV = G * D        # 256
NWL = 4           # windows per core
S = NWL * 128 + 128   # 640 input positions per core
OUT_ROWS = NWL * 256  # 1024 output rows per core
KC = C // 128     # 16 contraction chunks
NT = 4            # out-proj 512-col tiles
SC_OUT = OUT_ROWS // 128
SCALE = 1.0 / float(np.sqrt(D))
N_CORES = 8

_CACHE = {}


def _build():
    nc = bacc.Bacc(None, target_bir_lowering=False)

    # host-pretiled layouts (see kernel() for the numpy side)
    xT_d = nc.dram_tensor("xT", [128, KC, S], BF16, kind="ExternalInput")
    wq_d = nc.dram_tensor("Wq", [H, 128, KC, 128], BF16, kind="ExternalInput")
    wk_d = nc.dram_tensor("Wk", [G, 128, KC, 128], BF16, kind="ExternalInput")
    wv_d = nc.dram_tensor("Wv", [128, KC, KV], BF16, kind="ExternalInput")
    wo_d = nc.dram_tensor("Wo", [NT, 128, KC, 512], BF16, kind="ExternalInput")
    bq_d = nc.dram_tensor("bq", [C], F32, kind="ExternalInput")
    bk_d = nc.dram_tensor("bk", [KV], F32, kind="ExternalInput")
    bv_d = nc.dram_tensor("bv", [KV], F32, kind="ExternalInput")
    bo_d = nc.dram_tensor("bo", [C], F32, kind="ExternalInput")
    out_d = nc.dram_tensor("out", [NT, SC_OUT, 128, 512], F32,
                           kind="ExternalOutput")

    with tile.TileContext(nc) as tc:
        with (
            tc.tile_pool(name="res", bufs=1) as res,
            tc.tile_pool(name="wqs", bufs=3) as wqs,
            tc.tile_pool(name="wos", bufs=2) as wos,
            tc.tile_pool(name="work", bufs=3) as work,
            tc.tile_pool(name="norm", bufs=4) as norm,
            tc.tile_pool(name="dram", bufs=8, space="DRAM") as dpool,
        ):
            # ---------- resident loads ----------
            # first head's weights first so the PE can start ASAP, then x
            # in 4 chunks; small/secondary loads go on the gpsimd queue.
            wq0s = [res.tile([128, 4, 128], BF16, tag=f"wq0{j}", name=f"wq0{j}")
                    for j in range(4)]
            xtq = [res.tile([128, 2, S], BF16, tag=f"xt{j}", name=f"xt{j}")
                   for j in range(8)]
            nc.sync.dma_start(out=wq0s[0], in_=wq_d[0][:, 0:4, :])
            nc.sync.dma_start(out=xtq[0], in_=xT_d[:, 0:2, :])
            nc.sync.dma_start(out=xtq[1], in_=xT_d[:, 2:4, :])
            for j in range(1, 4):
                nc.sync.dma_start(out=wq0s[j], in_=wq_d[0][:, j * 4:(j + 1) * 4, :])
            for j in range(2, 8):
                nc.sync.dma_start(out=xtq[j], in_=xT_d[:, j * 2:(j + 1) * 2, :])

            def xts(kc):
                return xtq[kc // 2][:, kc % 2, :]

            wv_t = res.tile([128, KC, KV], BF16, tag="wv", name="wv")
            nc.gpsimd.dma_start(out=wv_t, in_=wv_d[:, :, :])
            kw = [res.tile([128, KC, 128], BF16, tag=f"kw{g}", name=f"kw{g}")
                  for g in range(G)]
            for g in range(G):
                nc.gpsimd.dma_start(out=kw[g], in_=wk_d[g])

            bq_sb = res.tile([128, H], F32, tag="bq", name="bq")
            nc.gpsimd.dma_start(out=bq_sb, in_=bq_d[:].rearrange("(h p) -> p h", p=128))
            bk_sb = res.tile([128, G], F32, tag="bk", name="bk")
            nc.gpsimd.dma_start(out=bk_sb, in_=bk_d[:].rearrange("(g p) -> p g", p=128))
            bv_bc = res.tile([128, KV], F32, tag="bvbc", name="bvbc")
            nc.gpsimd.dma_start(out=bv_bc,
                              in_=bv_d[:].unsqueeze(0).to_broadcast((128, KV)))
            bo_bc = res.tile([128, C], F32, tag="bobc", name="bobc")
            nc.gpsimd.dma_start(out=bo_bc,
                              in_=bo_d[:].unsqueeze(0).to_broadcast((128, C)))

            ones = res.tile([128, 1], BF16, tag="ones", name="ones")
            nc.vector.memset(ones, 1.0)
            ones_r = res.tile([1, 128], BF16, tag="ones_r", name="ones_r")
            nc.vector.memset(ones_r, 1.0)

            # paired Q storage: qp[g*4+j] holds heads (g+4j, g+4j+2)
            qp = [res.tile([128, 2, S], BF16, tag=f"qp{i}", name=f"qp{i}")
                  for i in range(8)]

            def q_slot(h):
                g, k = h % G, h // G
                return qp[g * 4 + k // 2][:, k % 2, :]
            kt = [res.tile([128, S], BF16, tag=f"kt{g}", name=f"kt{g}")
                  for g in range(G)]
            vt = [res.tile([128, KV], BF16, tag=f"vt{sc}", name=f"vt{sc}")
                  for sc in range(S // 128)]
            ot = [res.tile([128, OUT_ROWS], BF16, tag=f"ot{h}", name=f"ot{h}")
                  for h in range(H)]

            NA, NB = 320, 320  # free split of S=640 (psum bank = 512 f32)

            # ---------- projections ----------
            with tc.tile_pool(name="psA", bufs=2, space="PSUM") as psA:
                for h in range(H):
                    if h > 0:
                        wq_t = wqs.tile([128, KC, 128], BF16, tag="wq", name="wq")
                        nc.sync.dma_start(out=wq_t, in_=wq_d[h])

                    def wql(kc, h=h, wq_t=(None if h == 0 else wq_t)):
                        if h == 0:
                            return wq0s[kc // 4][:, kc % 4, :]
                        return wq_t[:, kc, :]
                    pa = psA.tile([128, NA], F32, tag="qa", name="qa")
                    pb = psA.tile([128, NB], F32, tag="qb", name="qb")
                    for kc in range(KC):
                        nc.tensor.matmul(pa, lhsT=wql(kc),
                                         rhs=xts(kc)[:, 0:NA],
                                         start=(kc == 0), stop=(kc == KC - 1))
                    for kc in range(KC):
                        nc.tensor.matmul(pb, lhsT=wql(kc),
                                         rhs=xts(kc)[:, NA:S],
                                         start=(kc == 0), stop=(kc == KC - 1))
                    nc.scalar.activation(q_slot(h)[:, 0:NA], pa,
                                         mybir.ActivationFunctionType.Identity,
                                         bias=bq_sb[:, h:h + 1])
                    nc.scalar.activation(q_slot(h)[:, NA:S], pb,
                                         mybir.ActivationFunctionType.Identity,
                                         bias=bq_sb[:, h:h + 1])

                for g in range(G):
                    pa = psA.tile([128, NA], F32, tag="qa", name="qa")
                    pb = psA.tile([128, NB], F32, tag="qb", name="qb")
                    for kc in range(KC):
                        nc.tensor.matmul(pa, lhsT=kw[g][:, kc, :],
                                         rhs=xts(kc)[:, 0:NA],
                                         start=(kc == 0), stop=(kc == KC - 1))
                    for kc in range(KC):
                        nc.tensor.matmul(pb, lhsT=kw[g][:, kc, :],
                                         rhs=xts(kc)[:, NA:S],
                                         start=(kc == 0), stop=(kc == KC - 1))
                    nc.scalar.activation(kt[g][:, 0:NA], pa,
                                         mybir.ActivationFunctionType.Identity,
                                         bias=bk_sb[:, g:g + 1])
                    nc.scalar.activation(kt[g][:, NA:S], pb,
                                         mybir.ActivationFunctionType.Identity,
                                         bias=bk_sb[:, g:g + 1])

                for sc in range(S // 128):
                    pv = psA.tile([128, KV], F32, tag="vp", name="vp")
                    for kc in range(KC):
                        nc.tensor.matmul(
                            pv, lhsT=xts(kc)[:, sc * 128:(sc + 1) * 128],
                            rhs=wv_t[:, kc, :],
                            start=(kc == 0), stop=(kc == KC - 1))
                    nc.vector.tensor_add(vt[sc], pv, bv_bc)

            # ---------- windowed attention (paired heads, pipelined) ----------
            # A "pair" = 2 heads of the same kv group -> N=512 matmuls over
            # both heads at once; 3-stage software pipeline keeps the PE from
            # waiting on ACT/DVE round-trips.
            pairs = [(w, g, j) for w in range(NWL)
                     for g in range(G) for j in range(4)]
            NP = len(pairs)
            state = {}

            def stage_a(i):
                w, g, j = pairs[i]
                q0 = w * 128
                qpt = qp[g * 4 + j]
                pt = work.tile([128, 2, 2, 256], BF16, tag="pt", name="pt")
                for kc in range(2):
                    st = psB.tile([128, 512], F32, tag="st", name="st")
                    nc.tensor.matmul(
                        st, lhsT=kt[g][:, q0 + kc * 128:q0 + (kc + 1) * 128],
                        rhs=qpt[:, :, q0:q0 + 256],
                        start=True, stop=True)
                    nc.scalar.activation(pt[:, kc, :, :], st,
                                         mybir.ActivationFunctionType.Exp,
                                         scale=SCALE)
                state[i] = [pt]

            def stage_b(i):
                w, g, j = pairs[i]
                (pt,) = state[i]
                ob = psB2.tile([128, 512], F32, tag="ob", name="ob")
                for kc in range(2):
                    nc.tensor.matmul(ob,
                                     lhsT=vt[w + kc][:, g * 128:(g + 1) * 128],
                                     rhs=pt[:, kc, :, :],
                                     start=(kc == 0), stop=(kc == 1))
                cs = psB3.tile([1, 512], F32, tag="cs", name="cs")
                for kc in range(2):
                    nc.tensor.matmul(cs, lhsT=ones, rhs=pt[:, kc, :, :],
                                     start=(kc == 0), stop=(kc == 1))
                csb = norm.tile([1, 512], BF16, tag="csb", name="csb")
                nc.scalar.copy(csb, cs)
                state[i] = [ob, csb]

            def stage_c(i):
                w, g, j = pairs[i]
                h0, h1 = g + 4 * j, g + 4 * j + 2
                ob, csb = state.pop(i)
                bc = psB3.tile([128, 512], F32, tag="bc", name="bc")
                nc.tensor.matmul(bc, lhsT=ones_r, rhs=csb,
                                 start=True, stop=True)
                bcr = norm.tile([128, 512], F32, tag="bcr", name="bcr")
                nc.vector.reciprocal_approx_fast(out=bcr, in_=bc)
                ws = slice(w * 256, (w + 1) * 256)
                nc.vector.tensor_mul(ot[h0][:, ws], ob[:, 0:256], bcr[:, 0:256])
                nc.vector.tensor_mul(ot[h1][:, ws], ob[:, 256:512], bcr[:, 256:512])

            with (
                tc.tile_pool(name="psB", bufs=4, space="PSUM") as psB,
                tc.tile_pool(name="psB2", bufs=2, space="PSUM") as psB2,
                tc.tile_pool(name="psB3", bufs=1, space="PSUM") as psB3,
            ):
                for i in range(NP + 2):
                    if 1 <= i < NP + 1:
                        stage_b(i - 1)
                    if i < NP:
                        stage_a(i)
                    if 2 <= i:
                        stage_c(i - 2)

            # ---------- out-projection ----------
            with tc.tile_pool(name="psC", bufs=4, space="PSUM") as psC:
                for nt in range(NT):
                    wo_t = wos.tile([128, KC, 512], BF16, tag="wo", name="wo")
                    nc.sync.dma_start(out=wo_t, in_=wo_d[nt])
                    for sc in range(SC_OUT):
                        po = psC.tile([128, 512], F32, tag="op", name="op")
                        for fc in range(KC):
                            nc.tensor.matmul(
                                po, lhsT=ot[fc][:, sc * 128:(sc + 1) * 128],
                                rhs=wo_t[:, fc, :],
                                start=(fc == 0), stop=(fc == KC - 1))
                        osb = work.tile([128, 512], F32, tag="osb", name="osb")
                        nc.vector.tensor_add(osb, po,
                                             bo_bc[:, nt * 512:(nt + 1) * 512])
                        nc.sync.dma_start(out=out_d[nt, sc], in_=osb)

    nc.compile()
    return nc


def _get_nc():
    if "nc" not in _CACHE:
        _CACHE["nc"] = _build()
    return _CACHE["nc"]


def _prep_weights(Wq, bq, Wk, bk, Wv, bv, Wo, bo):
    bf16 = ml_dtypes.bfloat16
    f32 = lambda a: np.ascontiguousarray(np.asarray(a, dtype=np.float32))
    wq = np.asarray(Wq, np.float32).reshape(KC, 128, H, 128)
    wq = np.ascontiguousarray(wq.transpose(2, 1, 0, 3)).astype(bf16)  # (H,p,kc,f)
    wk = np.asarray(Wk, np.float32).reshape(KC, 128, G, 128)
    wk = np.ascontiguousarray(wk.transpose(2, 1, 0, 3)).astype(bf16)  # (G,p,kc,f)
    wv = np.asarray(Wv, np.float32).reshape(KC, 128, KV)
    wv = np.ascontiguousarray(wv.transpose(1, 0, 2)).astype(bf16)     # (p,kc,f)
    wo = np.asarray(Wo, np.float32).reshape(KC, 128, NT, 512)
    wo = np.ascontiguousarray(wo.transpose(2, 1, 0, 3)).astype(bf16)  # (NT,p,kc,f)
    return {
        "Wq": wq, "Wk": wk, "Wv": wv, "Wo": wo,
        "bq": f32(bq), "bk": f32(bk), "bv": f32(bv), "bo": f32(bo),
    }


def _prep_in_maps(x, weights):
    bf16 = ml_dtypes.bfloat16
    in_maps = []
    for c in range(N_CORES):
        b, p = divmod(c, 4)
        xs = np.asarray(x[b, 512 * p:512 * p + S, :], np.float32)
        xT = np.ascontiguousarray(xs.T.reshape(KC, 128, S).transpose(1, 0, 2))
        in_maps.append(dict(weights, xT=xT.astype(bf16)))
    return in_maps


def kernel(x, Wq, bq, Wk, bk, Wv, bv, Wo, bo, **_):
    x = np.asarray(x, dtype=np.float32)
    weights = _prep_weights(Wq, bq, Wk, bk, Wv, bv, Wo, bo)
    in_maps = _prep_in_maps(x, weights)

    nc = _get_nc()
    res = run_bass_kernel_spmd(nc, in_maps, core_ids=list(range(N_CORES)))

    out = np.empty((B, L, C), dtype=np.float32)
    for c in range(N_CORES):
        b, p = divmod(c, 4)
        blk = res.results[c]["out"]  # (NT, SC_OUT, 128, 512)
        rows = blk.transpose(1, 2, 0, 3).reshape(OUT_ROWS, C)
        out[b, 1024 * p:1024 * p + OUT_ROWS, :] = rows
    return out
